# revision 6
# baseline (speedup 1.0000x reference)
"""Trainium2 Bass kernel for nn_NeuralDecisionTree.

Strategy (data-parallel over batch, 8 cores):
  reference:  x = features @ mask.T            [B, 1024]   (one-hot row select)
              d = sigmoid(x @ W + b)           [B, 1024]
              mu = tree-routing products       [B, 1024]
              out = mu @ softmax(pi)           [B, 100]

  The mask matmul is an exact column-selection, folded into W on the host.
  The host also pre-transposes/gathers features into [feature, batch] chunk
  layout and quantizes both matmul operands to fp8 e4m3 (W scaled by 16,
  descaled inside the fused sigmoid), so the device main matmul runs in
  DoubleRow fp8 mode: each MM contracts 256 features (two 128-row chunks
  packed per PE cell) at ~2x bf16 throughput.

    zT[s, b]  = sum_f W2p[f, s] * featT[f, b]             (PE, fp8 DoubleRow)
    d         = sigmoid(zT/16 + b)                        (ACT, bf16 out)
    mu        = 10 levels of routing products             (DVE, bf16;
                right child = mu - mu*d, so no second sigmoid is needed)
    yT[c, b]  = sum_s probsP[s, c] * mu10[s, b]           (PE, bf16)

  Node outputs are permuted on the host (slot permutation) so every tree
  level consumes a contiguous slice of d; levels 0-6 run in [batch, path]
  layout, levels 7-9 in [path-partition, batch] layout, and the leaf order
  is absorbed into a host-side row permutation of pi.  The two in-SBUF
  transposes (d-tile0 and mu7) run on the DMA xbar in bf16, keeping the PE
  stream pure matmul.

  Schedule: one fused PE stream per 512-row block —
    [tiles 1-4 MMs][tile0(next) MMs][tiles 5-7 MMs][8 leaf MMs]
  The tile0 group for block sg+1 sits mid-block so its sigmoid ->
  xbar-transpose -> DVE phase-A -> xbar-transpose chain (~4 us) finishes
  inside block sg, and the leaf MMs trail the DVE level-9 products by
  construction, so the PE never waits at a block boundary and the tail
  after the last main MM is just sig+mul+mm+sub+mm+copy+store.
"""

import ml_dtypes
import numpy as np

import concourse.bass as bass  # noqa: F401
import concourse.mybir as mybir
import concourse.tile as tile
from concourse import bacc
from concourse.bass_utils import run_bass_kernel_spmd

F32 = mybir.dt.float32
BF16 = mybir.dt.bfloat16
FP8 = mybir.dt.float8e4

B = 16384
NCORES = 8
BC = B // NCORES      # 2048 batch rows per core
SG = 512              # batch rows processed end-to-end per block
NSG = BC // SG        # 4
NF = 1024             # used features (host gathers mask-selected columns)
NL = 1024             # tree nodes / leaves / dense units
NCLS = 100            # classes
KCH = NF // 128       # 8 contraction chunks of 128
NDR = KCH // 2        # 4 double-row chunks of 256
NT = NL // 128        # 8 slot tiles
WSCALE = 16.0         # host premultiplies W2 by this; sigmoid descales
NWARM = 24            # PE warm-up matmuls covering the head DMA wait

# test.py can override (e.g. {"trace": True}) and read LAST_RESULT
RUN_KWARGS: dict = {}
LAST_RESULT = None


def _bitrev(q: int, bits: int) -> int:
    r = 0
    for m in range(bits):
        if (q >> m) & 1:
            r |= 1 << (bits - 1 - m)
    return r


def _node_of_slot() -> np.ndarray:
    """slot -> original node id. Slots are laid out so each tree level reads
    a contiguous [128, SG] slice of d at aligned partitions."""
    node = np.zeros(NL, dtype=np.int64)
    node[0] = 0  # unused slot (level-l nodes live at slots [2^l, 2^(l+1)),
    # so every phase-A slice starts at an even, 4B-aligned bf16 offset)
    for l in range(7):
        for q in range(1 << l):
            node[(1 << l) + q] = (1 << l) + _bitrev(q, l)
    for q7 in range(128):
        node[128 + q7] = 128 + _bitrev(q7, 7)
    for j1 in range(2):
        for q7 in range(128):
            node[256 + j1 * 128 + q7] = 256 + 2 * _bitrev(q7, 7) + j1
    for j2 in range(4):
        c7, c8 = j2 & 1, j2 >> 1
        for q7 in range(128):
            node[512 + j2 * 128 + q7] = 512 + 4 * _bitrev(q7, 7) + 2 * c7 + c8
    return node


def _leaf_of_row() -> np.ndarray:
    """probsP row r = j3*128 + q7 -> original leaf index."""
    L = np.zeros(NL, dtype=np.int64)
    for j3 in range(8):
        c789 = [j3 & 1, (j3 >> 1) & 1, (j3 >> 2) & 1]
        for q7 in range(128):
            c = [(q7 >> m) & 1 for m in range(7)] + c789
            L[j3 * 128 + q7] = sum(c[m] << (9 - m) for m in range(10))
    return L


def _build_program():
    nc = bacc.Bacc("TRN2", target_bir_lowering=False)
    feat = nc.dram_tensor("feat", [128, NSG * KCH * SG], FP8, kind="ExternalInput")
    w2p = nc.dram_tensor("w2p", [128, NT * NF], FP8, kind="ExternalInput")
    biases = nc.dram_tensor("biases", [128, 2 * NT], F32, kind="ExternalInput")
    pip = nc.dram_tensor("pip", [128, NT * NCLS], BF16, kind="ExternalInput")
    yT = nc.dram_tensor("yT", [NCLS, BC], F32, kind="ExternalOutput")

    SIG = mybir.ActivationFunctionType.Sigmoid
    DR = mybir.MatmulPerfMode.DoubleRow
    SGB = KCH * SG  # fp8 bytes per sg slice of feat, per partition
    QB = 2 * SG     # fp8 bytes per DR-chunk quarter, per partition

    with tile.TileContext(nc) as tc:
        with (
            tc.tile_pool(name="const", bufs=1) as cpool,
            tc.tile_pool(name="featT", bufs=3) as ftpool,
            tc.tile_pool(name="dsig", bufs=2) as dpool,
            tc.tile_pool(name="tree", bufs=2) as tpool,
            tc.tile_pool(name="mu", bufs=2) as mupool,
            tc.tile_pool(name="outst", bufs=2) as opool,
            tc.tile_pool(name="pw", bufs=1, space="PSUM") as pw,
            tc.tile_pool(name="pz", bufs=4, space="PSUM") as pz,
            tc.tile_pool(name="py", bufs=2, space="PSUM") as py,
        ):
            def load_ft0():
                """sg0 only: one tile per DoubleRow chunk so the first
                matmuls only depend on their own quarter's DMA."""
                fq = []
                for c in range(NDR):
                    q = ftpool.tile([128, QB], FP8, tag=f"ft{c}")
                    nc.sync.dma_start(q, feat[:, c * QB:(c + 1) * QB])
                    fq.append(q)
                return fq

            def load_ft(sg):
                """One dma_start for the whole sg slice, issued on the ACT
                hwdge queue so the SP queue carries only the xbar transposes
                (a DMA_TRANSPOSE waits for every in-flight DMA on its own
                queue, so input loads must stay off the transpose queue)."""
                big = ftpool.tile([128, SGB], FP8, tag="ftbig", bufs=3)
                nc.scalar.dma_start(big, feat[:, sg * SGB:(sg + 1) * SGB])
                return [big[:, c * QB:(c + 1) * QB] for c in range(NDR)]

            # ---- DMA priority order: the first block's critical loads on
            # the SP queue (their transfers finish before the first
            # transpose needs the queue); bulk tail loads on the ACT queue.
            w2 = cpool.tile([128, NT * NF], FP8)
            nc.sync.dma_start(w2[:, 0:NF], w2p[:, 0:NF])
            ft_bufs = {0: load_ft0()}
            bia = cpool.tile([128, 2 * NT], F32)
            nc.sync.dma_start(bia, biases[:, :])
            nc.sync.dma_start(w2[:, NF:4 * NF], w2p[:, NF:4 * NF])
            nc.scalar.dma_start(w2[:, 4 * NF:NT * NF], w2p[:, 4 * NF:NT * NF])
            pp = cpool.tile([128, NT * NCLS], BF16)
            nc.scalar.dma_start(pp, pip[:, :])
            ft_bufs[1] = load_ft(1)

            # warm-up burst: keep the PE busy during the head DMA wait so the
            # HAM clock gate is at 8/8 when the first real matmuls issue.
            wt = cpool.tile([128, 128], BF16)
            nc.gpsimd.memset(wt, 0.0)
            wp = pw.tile([128, 128], F32, tag="pt")
            for _ in range(NWARM):
                nc.tensor.matmul(wp, wt, wt, start=True, stop=True)

            ones = cpool.tile([128, 4], BF16)
            nc.gpsimd.memset(ones, 1.0)
            ones3 = ones.rearrange("p (u w) -> p u w", u=4)

            def mm_group(t, ft):
                """One slot tile's 4-chunk DoubleRow accumulation -> zp."""
                zp = pz.tile([128, SG], F32, tag="z")
                for c in range(NDR):
                    wsl = w2[:, (t * KCH + 2 * c) * 128:
                             (t * KCH + 2 * c + 2) * 128]
                    nc.tensor.matmul(
                        zp,
                        wsl.rearrange("p (k s) -> p k s", k=2),
                        ft[c].rearrange("p (k b) -> p k b", k=2),
                        start=(c == 0), stop=(c == NDR - 1),
                        perf_mode=DR,
                    )
                return zp

            def t0_group(ft):
                """Tile-0 matmul + sigmoid + t0T xbar transpose."""
                d0 = dpool.tile([128, SG], BF16, tag="d0")
                t0T = tpool.tile([128, 512], BF16, tag="t0T")
                zp = mm_group(0, ft)
                nc.scalar.activation(
                    d0, zp, SIG, bias=bia[:, 0:1], scale=1.0 / WSCALE
                )
                nc.sync.dma_start_transpose(
                    t0T.rearrange("p (u s) -> p u s", u=4), d0
                )
                return t0T

            def phase_a(t0T):
                """Tree levels 0-6 in [b, path] layout -> mu7 [b, 128]."""
                t03 = t0T.rearrange("p (u w) -> p u w", u=4)
                mu_prev = mupool.tile([128, 4 * 2], BF16, tag="muA1")
                mp3 = mu_prev.rearrange("p (u w) -> p u w", u=4)
                nc.vector.tensor_copy(mp3[:, :, 0:1], t03[:, :, 1:2])
                nc.vector.tensor_sub(mp3[:, :, 1:2], ones3, t03[:, :, 1:2])
                for l in range(1, 7):
                    w = 1 << l
                    mu_next = mupool.tile([128, 4 * 2 * w], BF16, tag=f"muA{l + 1}")
                    mn3 = mu_next.rearrange("p (u w) -> p u w", u=4)
                    nc.vector.tensor_mul(mn3[:, :, 0:w], mp3, t03[:, :, w:2 * w])
                    nc.vector.tensor_sub(mn3[:, :, w:2 * w], mp3, mn3[:, :, 0:w])
                    mu_prev, mp3 = mu_next, mn3
                return mu_prev

            def transpose_mu7(mu7):
                """mu7 -> [path-part, batch] via DMA xbar."""
                m7T = tpool.tile([128, 512], BF16, tag="m7T")
                nc.sync.dma_start_transpose(
                    m7T.rearrange("p (u q) -> p u q", u=4), mu7
                )
                return m7T

            # ---- prologue: tile0 chain for block 0 ----
            t0T_cur = t0_group(ft_bufs[0])
            m7T_cur = transpose_mu7(phase_a(t0T_cur))

            for sg in range(NSG):
                ft = ft_bufs.pop(sg)
                if sg + 2 < NSG:
                    ft_bufs[sg + 2] = load_ft(sg + 2)
                dsg = dpool.tile([128, 7 * SG], BF16, tag="d")

                # PE: tiles 1-4
                for t in range(1, 5):
                    zp = mm_group(t, ft)
                    nc.scalar.activation(
                        dsg[:, (t - 1) * SG:t * SG], zp, SIG,
                        bias=bia[:, t:t + 1], scale=1.0 / WSCALE,
                    )
                # PE: tile0 group for the NEXT block (mid-block so its
                # sig -> t0T -> phaseA -> m7T chain lands before block sg+1)
                t0T_next = (
                    t0_group(ft_bufs[sg + 1]) if sg + 1 < NSG else None
                )

                # DVE: tree levels 7-8 (ready early: m7T + tiles 1-3)
                mu8 = mupool.tile([128, 2 * SG], BF16, tag="mu8")
                nc.vector.tensor_mul(mu8[:, 0:SG], m7T_cur, dsg[:, 0:SG])
                nc.vector.tensor_sub(mu8[:, SG:2 * SG], m7T_cur, mu8[:, 0:SG])
                mu9 = mupool.tile([128, 4 * SG], BF16, tag="mu9")
                for j1 in range(2):
                    nc.vector.tensor_mul(
                        mu9[:, j1 * SG:(j1 + 1) * SG],
                        mu8[:, j1 * SG:(j1 + 1) * SG],
                        dsg[:, (1 + j1) * SG:(2 + j1) * SG],
                    )
                    nc.vector.tensor_sub(
                        mu9[:, (2 + j1) * SG:(3 + j1) * SG],
                        mu8[:, j1 * SG:(j1 + 1) * SG],
                        mu9[:, j1 * SG:(j1 + 1) * SG],
                    )

                # PE: tiles 5-7
                for t in range(5, NT):
                    zp = mm_group(t, ft)
                    nc.scalar.activation(
                        dsg[:, (t - 1) * SG:t * SG], zp, SIG,
                        bias=bia[:, t:t + 1], scale=1.0 / WSCALE,
                    )

                # DVE: tree level 9, ordered by d-tile availability
                mu10 = mupool.tile([128, 8 * SG], BF16, tag="mu10")
                for j2 in range(4):
                    nc.vector.tensor_mul(
                        mu10[:, j2 * SG:(j2 + 1) * SG],
                        mu9[:, j2 * SG:(j2 + 1) * SG],
                        dsg[:, (3 + j2) * SG:(4 + j2) * SG],
                    )
                    nc.vector.tensor_sub(
                        mu10[:, (4 + j2) * SG:(5 + j2) * SG],
                        mu9[:, j2 * SG:(j2 + 1) * SG],
                        mu10[:, j2 * SG:(j2 + 1) * SG],
                    )

                # PE: leaf matmuls, in mu10-readiness order
                yp = py.tile([NCLS, SG], F32, tag="y")
                leaf_order = [0, 4, 1, 5, 2, 6, 3, 7]
                for i, j3 in enumerate(leaf_order):
                    nc.tensor.matmul(
                        yp,
                        pp[:, j3 * NCLS:(j3 + 1) * NCLS],
                        mu10[:, j3 * SG:(j3 + 1) * SG],
                        start=(i == 0), stop=(i == 7),
                    )

                # DVE: phase A for block sg+1 (after mu10 so the FIFO never
                # stalls level-9 behind a t0T wait), then its transpose
                if t0T_next is not None:
                    m7T_cur = transpose_mu7(phase_a(t0T_next))

                ysb = opool.tile([NCLS, SG], F32, tag="ysb")
                nc.scalar.copy(ysb, yp)
                # store on the ACT hwdge queue: it trails the ysb copy in the
                # same FIFO and never delays feature loads on the SP queue
                nc.scalar.dma_start(yT[:, sg * SG:(sg + 1) * SG], ysb)

    nc.finalize()
    return nc


_PROGRAM = None


def _get_program():
    global _PROGRAM
    if _PROGRAM is None:
        _PROGRAM = _build_program()
    return _PROGRAM


def kernel(features, mask, W, b, pi):
    global LAST_RESULT
    features = np.asarray(features, dtype=np.float32)
    mask = np.asarray(mask)
    W = np.asarray(W, dtype=np.float32)
    b = np.asarray(b, dtype=np.float32)
    pi = np.asarray(pi, dtype=np.float32)

    # one-hot selection -> host column gather; apply slot/leaf permutations
    idx = np.argmax(mask, axis=1)
    node = _node_of_slot()
    W2p = W[:, node] * WSCALE
    w2p_resh = np.ascontiguousarray(
        W2p.reshape(KCH, 128, NT, 128).transpose(1, 2, 0, 3).reshape(128, NT * NF)
    )
    w2p_fp8 = np.clip(w2p_resh, -240.0, 240.0).astype(ml_dtypes.float8_e4m3fn)
    b2 = b[node].astype(np.float32)
    bcols = b2.reshape(NT, 128).T                      # [128, NT]
    biases = np.ascontiguousarray(
        np.concatenate([bcols, -bcols], axis=1), dtype=np.float32
    )
    e = np.exp(pi.astype(np.float64) - pi.max(1, keepdims=True))
    probs = (e / e.sum(1, keepdims=True)).astype(np.float32)
    piP = probs[_leaf_of_row(), :]
    pip_resh = np.ascontiguousarray(
        piP.reshape(NT, 128, NCLS).transpose(1, 0, 2).reshape(128, NT * NCLS)
    ).astype(ml_dtypes.bfloat16)
    feat_fp8 = np.clip(features[:, idx], -240.0, 240.0).astype(
        ml_dtypes.float8_e4m3fn
    )

    nc = _get_program()
    in_maps = []
    for c in range(NCORES):
        xc = feat_fp8[c * BC:(c + 1) * BC]            # [BC, NF]
        # device layout [p, sg, k, b]: feat[p, ...] = x[sg*SG+b, 128k+p]
        fdev = np.ascontiguousarray(
            xc.reshape(NSG, SG, KCH, 128).transpose(3, 0, 2, 1).reshape(128, -1)
        )
        in_maps.append(
            {"feat": fdev, "w2p": w2p_fp8, "biases": biases, "pip": pip_resh}
        )
    res = run_bass_kernel_spmd(nc, in_maps, core_ids=list(range(NCORES)), **RUN_KWARGS)
    LAST_RESULT = res
    yT_full = np.concatenate([res.results[c]["yT"] for c in range(NCORES)], axis=1)
    return np.ascontiguousarray(yT_full.T)


# revision 8
# speedup vs baseline: 1.1023x; 1.1023x over previous
"""Trainium2 Bass kernel for nn_NeuralDecisionTree.

Strategy (data-parallel over batch, 8 cores):
  reference:  x = features @ mask.T            [B, 1024]   (one-hot row select)
              d = sigmoid(x @ W + b)           [B, 1024]
              mu = tree-routing products       [B, 1024]
              out = mu @ softmax(pi)           [B, 100]

  The mask matmul is an exact column-selection, folded into W on the host.
  The host also pre-transposes/gathers features into [feature, batch] chunk
  layout and quantizes both matmul operands to fp8 e4m3 (W scaled by 16,
  descaled inside the fused sigmoid), so the device main matmul runs in
  DoubleRow fp8 mode: each MM contracts 256 features (two 128-row chunks
  packed per PE cell) at ~2x bf16 throughput.

    zT[s, b]  = sum_f W2p[f, s] * featT[f, b]             (PE, fp8 DoubleRow)
    d         = sigmoid(zT/16 + b)                        (ACT, bf16 out)
    mu        = 10 levels of routing products             (DVE, bf16;
                right child = mu - mu*d, so no second sigmoid is needed)
    yT[c, b]  = sum_s probsP[s, c] * mu10[s, b]           (PE, bf16)

  Node outputs are permuted on the host (slot permutation) so every tree
  level consumes a contiguous slice of d; levels 0-6 run in [batch, path]
  layout, levels 7-9 in [path-partition, batch] layout, and the leaf order
  is absorbed into a host-side row permutation of pi.  The two layout
  transposes (d-tile0 and mu7) run as PE chunk-transposes through an
  identity matrix into PSUM (a DMA_TRANSPOSE would drain every in-flight
  DMA, serializing against the input loads); the DVE reads PSUM directly.

  Schedule: one fused PE stream per 512-row block —
    [T1][m7T-tr][T2-4][T0(next)][T5-6][t0T(next)-tr][T7][8 leaf MMs]
  Each transpose sits in the PE stream just after its producer's sigmoid
  has had time to finish, the tile0 group for block sg+1 sits mid-block so
  its sigmoid -> transpose -> DVE phase-A chain completes inside block sg,
  and the leaf MMs trail the DVE level-9 products by construction, so the
  PE never waits at a block boundary and the tail after the last main MM
  is just sig+mul+mm+sub+mm+copy+store.
"""

import ml_dtypes
import numpy as np

import concourse.bass as bass  # noqa: F401
import concourse.mybir as mybir
import concourse.tile as tile
from concourse import bacc
from concourse.bass_utils import run_bass_kernel_spmd

F32 = mybir.dt.float32
BF16 = mybir.dt.bfloat16
FP8 = mybir.dt.float8e4

B = 16384
NCORES = 8
BC = B // NCORES      # 2048 batch rows per core
SG = 512              # batch rows processed end-to-end per block
NSG = BC // SG        # 4
NF = 1024             # used features (host gathers mask-selected columns)
NL = 1024             # tree nodes / leaves / dense units
NCLS = 100            # classes
KCH = NF // 128       # 8 contraction chunks of 128
NDR = KCH // 2        # 4 double-row chunks of 256
NT = NL // 128        # 8 slot tiles
WSCALE = 16.0         # host premultiplies W2 by this; sigmoid descales
NWARM = 24            # PE warm-up matmuls covering the head DMA wait

# test.py can override (e.g. {"trace": True}) and read LAST_RESULT
RUN_KWARGS: dict = {}
LAST_RESULT = None


def _bitrev(q: int, bits: int) -> int:
    r = 0
    for m in range(bits):
        if (q >> m) & 1:
            r |= 1 << (bits - 1 - m)
    return r


def _node_of_slot() -> np.ndarray:
    """slot -> original node id. Slots are laid out so each tree level reads
    a contiguous [128, SG] slice of d at aligned partitions."""
    node = np.zeros(NL, dtype=np.int64)
    node[0] = 0  # unused slot (level-l nodes live at slots [2^l, 2^(l+1)),
    # so every phase-A slice starts at an even, 4B-aligned bf16 offset)
    for l in range(7):
        for q in range(1 << l):
            node[(1 << l) + q] = (1 << l) + _bitrev(q, l)
    for q7 in range(128):
        node[128 + q7] = 128 + _bitrev(q7, 7)
    for j1 in range(2):
        for q7 in range(128):
            node[256 + j1 * 128 + q7] = 256 + 2 * _bitrev(q7, 7) + j1
    for j2 in range(4):
        c7, c8 = j2 & 1, j2 >> 1
        for q7 in range(128):
            node[512 + j2 * 128 + q7] = 512 + 4 * _bitrev(q7, 7) + 2 * c7 + c8
    return node


def _leaf_of_row() -> np.ndarray:
    """probsP row r = j3*128 + q7 -> original leaf index."""
    L = np.zeros(NL, dtype=np.int64)
    for j3 in range(8):
        c789 = [j3 & 1, (j3 >> 1) & 1, (j3 >> 2) & 1]
        for q7 in range(128):
            c = [(q7 >> m) & 1 for m in range(7)] + c789
            L[j3 * 128 + q7] = sum(c[m] << (9 - m) for m in range(10))
    return L


def _build_program():
    nc = bacc.Bacc("TRN2", target_bir_lowering=False)
    feat = nc.dram_tensor("feat", [128, NSG * KCH * SG], FP8, kind="ExternalInput")
    w2p = nc.dram_tensor("w2p", [128, NT * NF], FP8, kind="ExternalInput")
    biases = nc.dram_tensor("biases", [128, 2 * NT], F32, kind="ExternalInput")
    pip = nc.dram_tensor("pip", [128, NT * NCLS], BF16, kind="ExternalInput")
    idn = nc.dram_tensor("idn", [128, 128], BF16, kind="ExternalInput")
    yT = nc.dram_tensor("yT", [NCLS, BC], F32, kind="ExternalOutput")

    SIG = mybir.ActivationFunctionType.Sigmoid
    DR = mybir.MatmulPerfMode.DoubleRow
    SGB = KCH * SG  # fp8 bytes per sg slice of feat, per partition
    QB = 2 * SG     # fp8 bytes per DR-chunk quarter, per partition

    with tile.TileContext(nc) as tc:
        with (
            tc.tile_pool(name="const", bufs=1) as cpool,
            tc.tile_pool(name="featT", bufs=3) as ftpool,
            tc.tile_pool(name="dsig", bufs=2) as dpool,
            tc.tile_pool(name="mu", bufs=2) as mupool,
            tc.tile_pool(name="outst", bufs=2) as opool,
            tc.tile_pool(name="pz", bufs=4, space="PSUM") as pz,
            tc.tile_pool(name="py", bufs=2, space="PSUM") as py,
            tc.tile_pool(name="ptr", bufs=1, space="PSUM") as ptr,
        ):
            def load_ft0():
                """sg0 only: one tile per DoubleRow chunk so the first
                matmuls only depend on their own quarter's DMA."""
                fq = []
                for c in range(NDR):
                    q = ftpool.tile([128, QB], FP8, tag=f"ft{c}")
                    nc.sync.dma_start(q, feat[:, c * QB:(c + 1) * QB])
                    fq.append(q)
                return fq

            def load_ft(sg):
                """One dma_start for the whole sg slice (one SP issue slot);
                quarters are views into the one tile."""
                big = ftpool.tile([128, SGB], FP8, tag="ftbig", bufs=3)
                nc.sync.dma_start(big, feat[:, sg * SGB:(sg + 1) * SGB])
                return [big[:, c * QB:(c + 1) * QB] for c in range(NDR)]

            # ---- DMA priority order: everything the first block needs,
            # earliest-needed first (issue cost is ~0.6us per dma_start on
            # the SP sequencer, so keep the count low). ----
            w2 = cpool.tile([128, NT * NF], FP8)
            nc.sync.dma_start(w2[:, 0:NF], w2p[:, 0:NF])
            ft_bufs = {0: load_ft0()}
            bia = cpool.tile([128, 2 * NT], F32)
            nc.sync.dma_start(bia, biases[:, :])
            nc.sync.dma_start(w2[:, NF:4 * NF], w2p[:, NF:4 * NF])
            ident = cpool.tile([128, 128], BF16)
            nc.sync.dma_start(ident, idn[:, :])
            nc.sync.dma_start(w2[:, 4 * NF:NT * NF], w2p[:, 4 * NF:NT * NF])
            pp = cpool.tile([128, NT * NCLS], BF16)
            nc.sync.dma_start(pp, pip[:, :])
            ft_bufs[1] = load_ft(1)

            # warm-up burst: keep the PE busy during the head DMA wait so the
            # HAM clock gate is at 8/8 when the first real matmuls issue.
            wt = cpool.tile([128, 128], BF16)
            nc.gpsimd.memset(wt, 0.0)
            wp = pz.tile([128, SG], F32, tag="z")
            for _ in range(NWARM):
                nc.tensor.matmul(wp[:, 0:128], wt, wt, start=True, stop=True)

            ones = cpool.tile([128, 4], BF16)
            nc.gpsimd.memset(ones, 1.0)
            ones3 = ones.rearrange("p (u w) -> p u w", u=4)

            def mm_group(t, ft):
                """One slot tile's 4-chunk DoubleRow accumulation -> zp."""
                zp = pz.tile([128, SG], F32, tag="z")
                for c in range(NDR):
                    wsl = w2[:, (t * KCH + 2 * c) * 128:
                             (t * KCH + 2 * c + 2) * 128]
                    nc.tensor.matmul(
                        zp,
                        wsl.rearrange("p (k s) -> p k s", k=2),
                        ft[c].rearrange("p (k b) -> p k b", k=2),
                        start=(c == 0), stop=(c == NDR - 1),
                        perf_mode=DR,
                    )
                return zp

            def t0_group(ft):
                """Tile-0 matmul + sigmoid -> d0 [slot, b]."""
                d0 = dpool.tile([128, SG], BF16, tag="d0")
                zp = mm_group(0, ft)
                nc.scalar.activation(
                    d0, zp, SIG, bias=bia[:, 0:1], scale=1.0 / WSCALE
                )
                return d0

            def pe_transpose(dst_ps, src):
                """[128, 512] -> 4 PE chunk-transposes through ident:
                dst[p, u*128+s] = src[s, u*128+p]."""
                for u in range(4):
                    nc.tensor.transpose(
                        dst_ps[:, u * 128:(u + 1) * 128],
                        src[:, u * 128:(u + 1) * 128],
                        ident,
                    )

            def phase_a(t0T):
                """Tree levels 0-6 in [b, path] layout -> mu7 [b, 128]."""
                t03 = t0T.rearrange("p (u w) -> p u w", u=4)
                mu_prev = mupool.tile([128, 4 * 2], BF16, tag="muA1")
                mp3 = mu_prev.rearrange("p (u w) -> p u w", u=4)
                nc.vector.tensor_copy(mp3[:, :, 0:1], t03[:, :, 1:2])
                nc.vector.tensor_sub(mp3[:, :, 1:2], ones3, t03[:, :, 1:2])
                for l in range(1, 7):
                    w = 1 << l
                    mu_next = mupool.tile([128, 4 * 2 * w], BF16, tag=f"muA{l + 1}")
                    mn3 = mu_next.rearrange("p (u w) -> p u w", u=4)
                    nc.vector.tensor_mul(mn3[:, :, 0:w], mp3, t03[:, :, w:2 * w])
                    nc.vector.tensor_sub(mn3[:, :, w:2 * w], mp3, mn3[:, :, 0:w])
                    mu_prev, mp3 = mu_next, mn3
                return mu_prev

            # ---- prologue: tile0 chain for block 0 ----
            d0_cur = t0_group(ft_bufs[0])
            t0T_ps = ptr.tile([128, 512], BF16, tag="t0T")
            pe_transpose(t0T_ps, d0_cur)
            mu7_cur = phase_a(t0T_ps)

            for sg in range(NSG):
                ft = ft_bufs.pop(sg)
                if sg + 2 < NSG:
                    ft_bufs[sg + 2] = load_ft(sg + 2)
                dsg = dpool.tile([128, 7 * SG], BF16, tag="d")

                def tile_mm(t):
                    zp = mm_group(t, ft)
                    nc.scalar.activation(
                        dsg[:, (t - 1) * SG:t * SG], zp, SIG,
                        bias=bia[:, t:t + 1], scale=1.0 / WSCALE,
                    )

                # PE: tile 1, then mu7 -> m7T transpose (phase A of this
                # block finished by the end of the previous block)
                tile_mm(1)
                m7T_ps = ptr.tile([128, 512], BF16, tag="m7T")
                pe_transpose(m7T_ps, mu7_cur)
                for t in range(2, 5):
                    tile_mm(t)
                # PE: tile0 group for the NEXT block (mid-block so its
                # sigmoid -> transpose -> phase A chain lands in time)
                d0_next = t0_group(ft_bufs[sg + 1]) if sg + 1 < NSG else None

                # DVE: tree levels 7-8 (m7T PSUM + d tiles 1-3)
                mu8 = mupool.tile([128, 2 * SG], BF16, tag="mu8")
                nc.vector.tensor_mul(mu8[:, 0:SG], m7T_ps, dsg[:, 0:SG])
                nc.vector.tensor_sub(mu8[:, SG:2 * SG], m7T_ps, mu8[:, 0:SG])
                mu9 = mupool.tile([128, 4 * SG], BF16, tag="mu9")
                for j1 in range(2):
                    nc.vector.tensor_mul(
                        mu9[:, j1 * SG:(j1 + 1) * SG],
                        mu8[:, j1 * SG:(j1 + 1) * SG],
                        dsg[:, (1 + j1) * SG:(2 + j1) * SG],
                    )
                    nc.vector.tensor_sub(
                        mu9[:, (2 + j1) * SG:(3 + j1) * SG],
                        mu8[:, j1 * SG:(j1 + 1) * SG],
                        mu9[:, j1 * SG:(j1 + 1) * SG],
                    )

                # PE: tiles 5-6, then d0(next) -> t0T transpose (its sigmoid
                # ran during tiles 5-6), then tile 7
                tile_mm(5)
                tile_mm(6)
                if d0_next is not None:
                    t0T_ps = ptr.tile([128, 512], BF16, tag="t0T")
                    pe_transpose(t0T_ps, d0_next)
                tile_mm(7)

                # DVE: tree level 9, ordered by d-tile availability
                mu10 = mupool.tile([128, 8 * SG], BF16, tag="mu10")
                for j2 in range(4):
                    nc.vector.tensor_mul(
                        mu10[:, j2 * SG:(j2 + 1) * SG],
                        mu9[:, j2 * SG:(j2 + 1) * SG],
                        dsg[:, (3 + j2) * SG:(4 + j2) * SG],
                    )
                    nc.vector.tensor_sub(
                        mu10[:, (4 + j2) * SG:(5 + j2) * SG],
                        mu9[:, j2 * SG:(j2 + 1) * SG],
                        mu10[:, j2 * SG:(j2 + 1) * SG],
                    )

                # PE: leaf matmuls, in mu10-readiness order
                yp = py.tile([NCLS, SG], F32, tag="y")
                leaf_order = [0, 4, 1, 5, 2, 6, 3, 7]
                for i, j3 in enumerate(leaf_order):
                    nc.tensor.matmul(
                        yp,
                        pp[:, j3 * NCLS:(j3 + 1) * NCLS],
                        mu10[:, j3 * SG:(j3 + 1) * SG],
                        start=(i == 0), stop=(i == 7),
                    )

                # DVE: phase A for block sg+1 (after mu10 so the FIFO never
                # stalls level-9 behind the t0T transpose)
                if d0_next is not None:
                    mu7_cur = phase_a(t0T_ps)

                ysb = opool.tile([NCLS, SG], F32, tag="ysb")
                nc.scalar.copy(ysb, yp)
                # store on the ACT hwdge queue: it trails the ysb copy in the
                # same FIFO and never delays feature loads on the SP queue
                nc.scalar.dma_start(yT[:, sg * SG:(sg + 1) * SG], ysb)

    nc.finalize()
    return nc


_PROGRAM = None


def _get_program():
    global _PROGRAM
    if _PROGRAM is None:
        _PROGRAM = _build_program()
    return _PROGRAM


def kernel(features, mask, W, b, pi):
    global LAST_RESULT
    features = np.asarray(features, dtype=np.float32)
    mask = np.asarray(mask)
    W = np.asarray(W, dtype=np.float32)
    b = np.asarray(b, dtype=np.float32)
    pi = np.asarray(pi, dtype=np.float32)

    # one-hot selection -> host column gather; apply slot/leaf permutations
    idx = np.argmax(mask, axis=1)
    node = _node_of_slot()
    W2p = W[:, node] * WSCALE
    w2p_resh = np.ascontiguousarray(
        W2p.reshape(KCH, 128, NT, 128).transpose(1, 2, 0, 3).reshape(128, NT * NF)
    )
    w2p_fp8 = np.clip(w2p_resh, -240.0, 240.0).astype(ml_dtypes.float8_e4m3fn)
    b2 = b[node].astype(np.float32)
    bcols = b2.reshape(NT, 128).T                      # [128, NT]
    biases = np.ascontiguousarray(
        np.concatenate([bcols, -bcols], axis=1), dtype=np.float32
    )
    e = np.exp(pi.astype(np.float64) - pi.max(1, keepdims=True))
    probs = (e / e.sum(1, keepdims=True)).astype(np.float32)
    piP = probs[_leaf_of_row(), :]
    pip_resh = np.ascontiguousarray(
        piP.reshape(NT, 128, NCLS).transpose(1, 0, 2).reshape(128, NT * NCLS)
    ).astype(ml_dtypes.bfloat16)
    feat_fp8 = np.clip(features[:, idx], -240.0, 240.0).astype(
        ml_dtypes.float8_e4m3fn
    )
    ident = np.eye(128, dtype=ml_dtypes.bfloat16)

    nc = _get_program()
    in_maps = []
    for c in range(NCORES):
        xc = feat_fp8[c * BC:(c + 1) * BC]            # [BC, NF]
        # device layout [p, sg, k, b]: feat[p, ...] = x[sg*SG+b, 128k+p]
        fdev = np.ascontiguousarray(
            xc.reshape(NSG, SG, KCH, 128).transpose(3, 0, 2, 1).reshape(128, -1)
        )
        in_maps.append(
            {"feat": fdev, "w2p": w2p_fp8, "biases": biases, "pip": pip_resh,
             "idn": ident}
        )
    res = run_bass_kernel_spmd(nc, in_maps, core_ids=list(range(NCORES)), **RUN_KWARGS)
    LAST_RESULT = res
    yT_full = np.concatenate([res.results[c]["yT"] for c in range(NCORES)], axis=1)
    return np.ascontiguousarray(yT_full.T)


# revision 10
# speedup vs baseline: 1.1267x; 1.0222x over previous
"""Trainium2 Bass kernel for nn_NeuralDecisionTree.

Strategy (data-parallel over batch, 8 cores):
  reference:  x = features @ mask.T            [B, 1024]   (one-hot row select)
              d = sigmoid(x @ W + b)           [B, 1024]
              mu = tree-routing products       [B, 1024]
              out = mu @ softmax(pi)           [B, 100]

  The mask matmul is an exact column-selection, folded into W on the host.
  The host also pre-transposes/gathers features into [feature, batch] chunk
  layout and quantizes both matmul operands to fp8 e4m3 (W scaled by 16,
  descaled inside the fused sigmoid), so the device main matmul runs in
  DoubleRow fp8 mode: each MM contracts 256 features (two 128-row chunks
  packed per PE cell) at ~2x bf16 throughput.

    zT[s, b]  = sum_f W2p[f, s] * featT[f, b]             (PE, fp8 DoubleRow)
    d         = sigmoid(zT/16 + b)                        (ACT, bf16 out)
    mu        = 10 levels of routing products             (DVE, bf16;
                right child = mu - mu*d, so no second sigmoid is needed)
    yT[c, b]  = sum_s probsP[s, c] * mu10[s, b]           (PE, bf16)

  Node outputs are permuted on the host (slot permutation) so every tree
  level consumes a contiguous slice of d; levels 0-6 run in [batch, path]
  layout, levels 7-9 in [path-partition, batch] layout, and the leaf order
  is absorbed into a host-side row permutation of pi.  The two layout
  transposes (d-tile0 and mu7) run as PE chunk-transposes through an
  identity matrix into PSUM (a DMA_TRANSPOSE would drain every in-flight
  DMA, serializing against the input loads); the DVE reads PSUM directly.

  Schedule: one fused PE stream per 512-row block —
    [T1][m7T-tr][T2-4][T0(next)][T5-6][t0T(next)-tr][T7][8 leaf MMs]
  Each transpose sits in the PE stream just after its producer's sigmoid
  has had time to finish, the tile0 group for block sg+1 sits mid-block so
  its sigmoid -> transpose -> DVE phase-A chain completes inside block sg,
  and the leaf MMs trail the DVE level-9 products by construction, so the
  PE never waits at a block boundary and the tail after the last main MM
  is just sig+mul+mm+sub+mm+copy+store.
"""

import ml_dtypes
import numpy as np

import concourse.bass as bass  # noqa: F401
import concourse.mybir as mybir
import concourse.tile as tile
from concourse import bacc
from concourse.bass_utils import run_bass_kernel_spmd

F32 = mybir.dt.float32
BF16 = mybir.dt.bfloat16
FP8 = mybir.dt.float8e4

B = 16384
NCORES = 8
BC = B // NCORES      # 2048 batch rows per core
SG = 512              # batch rows processed end-to-end per block
NSG = BC // SG        # 4
NF = 1024             # used features (host gathers mask-selected columns)
NL = 1024             # tree nodes / leaves / dense units
NCLS = 100            # classes
KCH = NF // 128       # 8 contraction chunks of 128
NDR = KCH // 2        # 4 double-row chunks of 256
NT = NL // 128        # 8 slot tiles
WSCALE = 16.0         # host premultiplies W2 by this; sigmoid descales
NWARM = 24            # PE warm-up matmuls covering the head DMA wait

# test.py can override (e.g. {"trace": True}) and read LAST_RESULT
RUN_KWARGS: dict = {}
LAST_RESULT = None


def _bitrev(q: int, bits: int) -> int:
    r = 0
    for m in range(bits):
        if (q >> m) & 1:
            r |= 1 << (bits - 1 - m)
    return r


def _node_of_slot() -> np.ndarray:
    """slot -> original node id. Slots are laid out so each tree level reads
    a contiguous [128, SG] slice of d at aligned partitions."""
    node = np.zeros(NL, dtype=np.int64)
    node[0] = 0  # unused slot (level-l nodes live at slots [2^l, 2^(l+1)),
    # so every phase-A slice starts at an even, 4B-aligned bf16 offset)
    for l in range(7):
        for q in range(1 << l):
            node[(1 << l) + q] = (1 << l) + _bitrev(q, l)
    for q7 in range(128):
        node[128 + q7] = 128 + _bitrev(q7, 7)
    for j1 in range(2):
        for q7 in range(128):
            node[256 + j1 * 128 + q7] = 256 + 2 * _bitrev(q7, 7) + j1
    for j2 in range(4):
        c7, c8 = j2 & 1, j2 >> 1
        for q7 in range(128):
            node[512 + j2 * 128 + q7] = 512 + 4 * _bitrev(q7, 7) + 2 * c7 + c8
    return node


def _leaf_of_row() -> np.ndarray:
    """probsP row r = j3*128 + q7 -> original leaf index."""
    L = np.zeros(NL, dtype=np.int64)
    for j3 in range(8):
        c789 = [j3 & 1, (j3 >> 1) & 1, (j3 >> 2) & 1]
        for q7 in range(128):
            c = [(q7 >> m) & 1 for m in range(7)] + c789
            L[j3 * 128 + q7] = sum(c[m] << (9 - m) for m in range(10))
    return L


def _build_program():
    nc = bacc.Bacc("TRN2", target_bir_lowering=False)
    feat = nc.dram_tensor("feat", [128, NSG * KCH * SG], FP8, kind="ExternalInput")
    w2p = nc.dram_tensor("w2p", [128, NT * NF], FP8, kind="ExternalInput")
    biases = nc.dram_tensor("biases", [128, 2 * NT], F32, kind="ExternalInput")
    pip = nc.dram_tensor("pip", [128, NT * NCLS], BF16, kind="ExternalInput")
    yT = nc.dram_tensor("yT", [NCLS, BC], F32, kind="ExternalOutput")

    SIG = mybir.ActivationFunctionType.Sigmoid
    DR = mybir.MatmulPerfMode.DoubleRow
    SGB = KCH * SG  # fp8 bytes per sg slice of feat, per partition
    QB = 2 * SG     # fp8 bytes per DR-chunk quarter, per partition

    with tile.TileContext(nc) as tc:
        with (
            tc.tile_pool(name="const", bufs=1) as cpool,
            tc.tile_pool(name="featT", bufs=3) as ftpool,
            tc.tile_pool(name="dsig", bufs=2) as dpool,
            tc.tile_pool(name="mu", bufs=2) as mupool,
            tc.tile_pool(name="outst", bufs=2) as opool,
            tc.tile_pool(name="tree", bufs=2) as tpool,
            tc.tile_pool(name="pz", bufs=4, space="PSUM") as pz,
            tc.tile_pool(name="py", bufs=2, space="PSUM") as py,
        ):

            def load_ft(sg):
                """One dma_start for the whole sg slice (one SP issue slot);
                quarters are views into the one tile."""
                big = ftpool.tile([128, SGB], FP8, tag="ftbig", bufs=3)
                nc.sync.dma_start(big, feat[:, sg * SGB:(sg + 1) * SGB])
                return [big[:, c * QB:(c + 1) * QB] for c in range(NDR)]

            # ---- DMA priority order: everything the first block needs,
            # earliest-needed first (issue cost is ~0.6us per dma_start on
            # the SP sequencer, so keep the count low). ----
            w2 = cpool.tile([128, NT * NF], FP8)
            nc.sync.dma_start(w2[:, 0:NF], w2p[:, 0:NF])
            ft_bufs = {0: load_ft(0)}
            bia = cpool.tile([128, 2 * NT], F32)
            nc.sync.dma_start(bia, biases[:, :])
            ft_bufs[1] = load_ft(1)
            # bulk loads ride the GpSimd SWDGE queue: they issue in parallel
            # with the SP queue and keep the SP drain-set small for the
            # first transposes
            nc.gpsimd.dma_start(w2[:, NF:4 * NF], w2p[:, NF:4 * NF])
            nc.gpsimd.dma_start(w2[:, 4 * NF:NT * NF], w2p[:, 4 * NF:NT * NF])
            pp = cpool.tile([128, NT * NCLS], BF16)
            nc.gpsimd.dma_start(pp, pip[:, :])

            # warm-up burst: keep the PE busy during the head DMA wait so the
            # HAM clock gate is at 8/8 when the first real matmuls issue.
            wt = cpool.tile([128, 128], BF16)
            nc.gpsimd.memset(wt, 0.0)
            wp = pz.tile([128, SG], F32, tag="z")
            for _ in range(NWARM):
                nc.tensor.matmul(wp[:, 0:128], wt, wt, start=True, stop=True)

            ones = cpool.tile([128, 4], BF16)
            nc.gpsimd.memset(ones, 1.0)
            ones3 = ones.rearrange("p (u w) -> p u w", u=4)

            def mm_group(t, ft):
                """One slot tile's 4-chunk DoubleRow accumulation -> zp."""
                zp = pz.tile([128, SG], F32, tag="z")
                for c in range(NDR):
                    wsl = w2[:, (t * KCH + 2 * c) * 128:
                             (t * KCH + 2 * c + 2) * 128]
                    nc.tensor.matmul(
                        zp,
                        wsl.rearrange("p (k s) -> p k s", k=2),
                        ft[c].rearrange("p (k b) -> p k b", k=2),
                        start=(c == 0), stop=(c == NDR - 1),
                        perf_mode=DR,
                    )
                return zp

            def t0_group(ft):
                """Tile-0 matmul + sigmoid -> d0 [slot, b]."""
                d0 = dpool.tile([128, SG], BF16, tag="d0")
                zp = mm_group(0, ft)
                nc.scalar.activation(
                    d0, zp, SIG, bias=bia[:, 0:1], scale=1.0 / WSCALE
                )
                return d0

            def xbar_transpose(tag, src):
                """[128, 512] -> chunkwise DMA-xbar transpose:
                dst[p, u*128+s] = src[s, u*128+p].  NOTE a DMA_TRANSPOSE
                drains every in-flight DMA, so SP-queue emission order
                decides what it ends up waiting for."""
                dst = tpool.tile([128, 512], BF16, tag=tag)
                nc.sync.dma_start_transpose(
                    dst.rearrange("p (u s) -> p u s", u=4), src
                )
                return dst

            def phase_a(t0T):
                """Tree levels 0-6 in [b, path] layout -> mu7 [b, 128]."""
                t03 = t0T.rearrange("p (u w) -> p u w", u=4)
                mu_prev = mupool.tile([128, 4 * 2], BF16, tag="muA1")
                mp3 = mu_prev.rearrange("p (u w) -> p u w", u=4)
                nc.vector.tensor_copy(mp3[:, :, 0:1], t03[:, :, 1:2])
                nc.vector.tensor_sub(mp3[:, :, 1:2], ones3, t03[:, :, 1:2])
                for l in range(1, 7):
                    w = 1 << l
                    mu_next = mupool.tile([128, 4 * 2 * w], BF16, tag=f"muA{l + 1}")
                    mn3 = mu_next.rearrange("p (u w) -> p u w", u=4)
                    nc.vector.tensor_mul(mn3[:, :, 0:w], mp3, t03[:, :, w:2 * w])
                    nc.vector.tensor_sub(mn3[:, :, w:2 * w], mp3, mn3[:, :, 0:w])
                    mu_prev, mp3 = mu_next, mn3
                return mu_prev

            # ---- prologue: tile0 chain for block 0 ----
            d0_cur = t0_group(ft_bufs[0])
            mu7_cur = phase_a(xbar_transpose("t0T", d0_cur))

            for sg in range(NSG):
                ft = ft_bufs.pop(sg)
                # SP order: m7T(sg) first (its inputs finished last block and
                # the older in-flight DMAs are done, so its drain is cheap),
                # THEN the next feature load
                m7T = xbar_transpose("m7T", mu7_cur)
                if sg + 2 < NSG:
                    ft_bufs[sg + 2] = load_ft(sg + 2)
                dsg = dpool.tile([128, 7 * SG], BF16, tag="d")

                def tile_mm(t):
                    zp = mm_group(t, ft)
                    nc.scalar.activation(
                        dsg[:, (t - 1) * SG:t * SG], zp, SIG,
                        bias=bia[:, t:t + 1], scale=1.0 / WSCALE,
                    )

                for t in range(1, 5):
                    tile_mm(t)
                # PE: tile0 group for the NEXT block (mid-block so its
                # sigmoid -> transpose -> phase A chain lands in time)
                d0_next = t0_group(ft_bufs[sg + 1]) if sg + 1 < NSG else None

                # DVE: tree levels 7-8 (m7T PSUM + d tiles 1-3)
                mu8 = mupool.tile([128, 2 * SG], BF16, tag="mu8")
                nc.vector.tensor_mul(mu8[:, 0:SG], m7T, dsg[:, 0:SG])
                nc.vector.tensor_sub(mu8[:, SG:2 * SG], m7T, mu8[:, 0:SG])
                mu9 = mupool.tile([128, 4 * SG], BF16, tag="mu9")
                for j1 in range(2):
                    nc.vector.tensor_mul(
                        mu9[:, j1 * SG:(j1 + 1) * SG],
                        mu8[:, j1 * SG:(j1 + 1) * SG],
                        dsg[:, (1 + j1) * SG:(2 + j1) * SG],
                    )
                    nc.vector.tensor_sub(
                        mu9[:, (2 + j1) * SG:(3 + j1) * SG],
                        mu8[:, j1 * SG:(j1 + 1) * SG],
                        mu9[:, j1 * SG:(j1 + 1) * SG],
                    )

                # SP: t0T(next) transpose (drain set: this block's own
                # feature load, already landed)
                t0T_next = (
                    xbar_transpose("t0T", d0_next) if d0_next is not None
                    else None
                )
                tile_mm(5)
                tile_mm(6)
                tile_mm(7)

                # DVE: tree level 9, ordered by d-tile availability
                mu10 = mupool.tile([128, 8 * SG], BF16, tag="mu10")
                for j2 in range(4):
                    nc.vector.tensor_mul(
                        mu10[:, j2 * SG:(j2 + 1) * SG],
                        mu9[:, j2 * SG:(j2 + 1) * SG],
                        dsg[:, (3 + j2) * SG:(4 + j2) * SG],
                    )
                    nc.vector.tensor_sub(
                        mu10[:, (4 + j2) * SG:(5 + j2) * SG],
                        mu9[:, j2 * SG:(j2 + 1) * SG],
                        mu10[:, j2 * SG:(j2 + 1) * SG],
                    )

                # PE: leaf matmuls, in mu10-readiness order
                yp = py.tile([NCLS, SG], F32, tag="y")
                leaf_order = [0, 4, 1, 5, 2, 6, 3, 7]
                for i, j3 in enumerate(leaf_order):
                    nc.tensor.matmul(
                        yp,
                        pp[:, j3 * NCLS:(j3 + 1) * NCLS],
                        mu10[:, j3 * SG:(j3 + 1) * SG],
                        start=(i == 0), stop=(i == 7),
                    )

                # DVE: phase A for block sg+1 (after mu10 so the FIFO never
                # stalls level-9 behind the t0T transpose)
                if t0T_next is not None:
                    mu7_cur = phase_a(t0T_next)

                ysb = opool.tile([NCLS, SG], F32, tag="ysb")
                nc.scalar.copy(ysb, yp)
                # store on the ACT hwdge queue: it trails the ysb copy in the
                # same FIFO and never delays feature loads on the SP queue
                nc.scalar.dma_start(yT[:, sg * SG:(sg + 1) * SG], ysb)

    nc.finalize()
    return nc


_PROGRAM = None


def _get_program():
    global _PROGRAM
    if _PROGRAM is None:
        _PROGRAM = _build_program()
    return _PROGRAM


def kernel(features, mask, W, b, pi):
    global LAST_RESULT
    features = np.asarray(features, dtype=np.float32)
    mask = np.asarray(mask)
    W = np.asarray(W, dtype=np.float32)
    b = np.asarray(b, dtype=np.float32)
    pi = np.asarray(pi, dtype=np.float32)

    # one-hot selection -> host column gather; apply slot/leaf permutations
    idx = np.argmax(mask, axis=1)
    node = _node_of_slot()
    W2p = W[:, node] * WSCALE
    w2p_resh = np.ascontiguousarray(
        W2p.reshape(KCH, 128, NT, 128).transpose(1, 2, 0, 3).reshape(128, NT * NF)
    )
    w2p_fp8 = np.clip(w2p_resh, -240.0, 240.0).astype(ml_dtypes.float8_e4m3fn)
    b2 = b[node].astype(np.float32)
    bcols = b2.reshape(NT, 128).T                      # [128, NT]
    biases = np.ascontiguousarray(
        np.concatenate([bcols, -bcols], axis=1), dtype=np.float32
    )
    e = np.exp(pi.astype(np.float64) - pi.max(1, keepdims=True))
    probs = (e / e.sum(1, keepdims=True)).astype(np.float32)
    piP = probs[_leaf_of_row(), :]
    pip_resh = np.ascontiguousarray(
        piP.reshape(NT, 128, NCLS).transpose(1, 0, 2).reshape(128, NT * NCLS)
    ).astype(ml_dtypes.bfloat16)
    feat_fp8 = np.clip(features[:, idx], -240.0, 240.0).astype(
        ml_dtypes.float8_e4m3fn
    )

    nc = _get_program()
    in_maps = []
    for c in range(NCORES):
        xc = feat_fp8[c * BC:(c + 1) * BC]            # [BC, NF]
        # device layout [p, sg, k, b]: feat[p, ...] = x[sg*SG+b, 128k+p]
        fdev = np.ascontiguousarray(
            xc.reshape(NSG, SG, KCH, 128).transpose(3, 0, 2, 1).reshape(128, -1)
        )
        in_maps.append(
            {"feat": fdev, "w2p": w2p_fp8, "biases": biases, "pip": pip_resh}
        )
    res = run_bass_kernel_spmd(nc, in_maps, core_ids=list(range(NCORES)), **RUN_KWARGS)
    LAST_RESULT = res
    yT_full = np.concatenate([res.results[c]["yT"] for c in range(NCORES)], axis=1)
    return np.ascontiguousarray(yT_full.T)


# revision 12
# speedup vs baseline: 1.1778x; 1.0453x over previous
"""Trainium2 Bass kernel for nn_NeuralDecisionTree.

Strategy (data-parallel over batch, 8 cores):
  reference:  x = features @ mask.T            [B, 1024]   (one-hot row select)
              d = sigmoid(x @ W + b)           [B, 1024]
              mu = tree-routing products       [B, 1024]
              out = mu @ softmax(pi)           [B, 100]

  The mask matmul is an exact column-selection, folded into W on the host.
  The host also pre-transposes/gathers features into [feature, batch] chunk
  layout and quantizes both matmul operands to fp8 e4m3 (W scaled by 16,
  descaled inside the fused sigmoid), so the device main matmul runs in
  DoubleRow fp8 mode: each MM contracts 256 features (two 128-row chunks
  packed per PE cell) at ~2x bf16 throughput.

    zT[s, b]  = sum_f W2p[f, s] * featT[f, b]             (PE, fp8 DoubleRow)
    d         = sigmoid(zT/16 + b)                        (ACT, bf16 out)
    mu        = 10 levels of routing products             (DVE, bf16;
                right child = mu - mu*d, so no second sigmoid is needed)
    yT[c, b]  = sum_s probsP[s, c] * mu10[s, b]           (PE, bf16)

  Node outputs are permuted on the host (slot permutation) so every tree
  level consumes a contiguous slice of d; levels 0-6 run in [batch, path]
  layout, levels 7-9 in [path-partition, batch] layout, and the leaf order
  is absorbed into a host-side row permutation of pi.  The two layout
  transposes (d-tile0 and mu7) run as PE chunk-transposes through an
  identity matrix into PSUM (a DMA_TRANSPOSE would drain every in-flight
  DMA, serializing against the input loads); the DVE reads PSUM directly.

  Schedule: one fused PE stream per 512-row block —
    [T1][m7T-tr][T2-4][T0(next)][T5-6][t0T(next)-tr][T7][8 leaf MMs]
  Each transpose sits in the PE stream just after its producer's sigmoid
  has had time to finish, the tile0 group for block sg+1 sits mid-block so
  its sigmoid -> transpose -> DVE phase-A chain completes inside block sg,
  and the leaf MMs trail the DVE level-9 products by construction, so the
  PE never waits at a block boundary and the tail after the last main MM
  is just sig+mul+mm+sub+mm+copy+store.
"""

import ml_dtypes
import numpy as np

import concourse.bass as bass  # noqa: F401
import concourse.mybir as mybir
import concourse.tile as tile
from concourse import bacc
from concourse.bass_utils import run_bass_kernel_spmd

F32 = mybir.dt.float32
BF16 = mybir.dt.bfloat16
FP8 = mybir.dt.float8e4

B = 16384
NCORES = 8
BC = B // NCORES      # 2048 batch rows per core
SG = 512              # batch rows processed end-to-end per block
NSG = BC // SG        # 4
NF = 1024             # used features (host gathers mask-selected columns)
NL = 1024             # tree nodes / leaves / dense units
NCLS = 100            # classes
KCH = NF // 128       # 8 contraction chunks of 128
NDR = KCH // 2        # 4 double-row chunks of 256
NT = NL // 128        # 8 slot tiles
WSCALE = 16.0         # host premultiplies W2 by this; sigmoid descales
NWARM = 36            # PE warm-up matmuls covering the head DMA wait

# test.py can override (e.g. {"trace": True}) and read LAST_RESULT
RUN_KWARGS: dict = {}
LAST_RESULT = None


def _bitrev(q: int, bits: int) -> int:
    r = 0
    for m in range(bits):
        if (q >> m) & 1:
            r |= 1 << (bits - 1 - m)
    return r


def _node_of_slot() -> np.ndarray:
    """slot -> original node id. Slots are laid out so each tree level reads
    a contiguous [128, SG] slice of d at aligned partitions."""
    node = np.zeros(NL, dtype=np.int64)
    node[0] = 0  # unused slot (level-l nodes live at slots [2^l, 2^(l+1)),
    # so every phase-A slice starts at an even, 4B-aligned bf16 offset)
    for l in range(7):
        for q in range(1 << l):
            node[(1 << l) + q] = (1 << l) + _bitrev(q, l)
    for q7 in range(128):
        node[128 + q7] = 128 + _bitrev(q7, 7)
    for j1 in range(2):
        for q7 in range(128):
            node[256 + j1 * 128 + q7] = 256 + 2 * _bitrev(q7, 7) + j1
    for j2 in range(4):
        c7, c8 = j2 & 1, j2 >> 1
        for q7 in range(128):
            node[512 + j2 * 128 + q7] = 512 + 4 * _bitrev(q7, 7) + 2 * c7 + c8
    return node


def _leaf_of_row() -> np.ndarray:
    """probsP row r = j3*128 + q7 -> original leaf index."""
    L = np.zeros(NL, dtype=np.int64)
    for j3 in range(8):
        c789 = [j3 & 1, (j3 >> 1) & 1, (j3 >> 2) & 1]
        for q7 in range(128):
            c = [(q7 >> m) & 1 for m in range(7)] + c789
            L[j3 * 128 + q7] = sum(c[m] << (9 - m) for m in range(10))
    return L


def _build_program():
    nc = bacc.Bacc("TRN2", target_bir_lowering=False)
    feat = nc.dram_tensor("feat", [128, NSG * KCH * SG], FP8, kind="ExternalInput")
    w2p = nc.dram_tensor("w2p", [128, NT * NF], FP8, kind="ExternalInput")
    biases = nc.dram_tensor("biases", [128, 2 * NT], F32, kind="ExternalInput")
    pip = nc.dram_tensor("pip", [128, NT * NCLS], BF16, kind="ExternalInput")
    yT = nc.dram_tensor("yT", [NCLS, BC], F32, kind="ExternalOutput")

    SIG = mybir.ActivationFunctionType.Sigmoid
    DR = mybir.MatmulPerfMode.DoubleRow
    SGB = KCH * SG  # fp8 bytes per sg slice of feat, per partition
    QB = 2 * SG     # fp8 bytes per DR-chunk quarter, per partition

    with tile.TileContext(nc) as tc:
        with (
            tc.tile_pool(name="const", bufs=1) as cpool,
            tc.tile_pool(name="featT", bufs=3) as ftpool,
            tc.tile_pool(name="dsig", bufs=2) as dpool,
            tc.tile_pool(name="mu", bufs=2) as mupool,
            tc.tile_pool(name="outst", bufs=2) as opool,
            tc.tile_pool(name="tree", bufs=2) as tpool,
            tc.tile_pool(name="pz", bufs=4, space="PSUM") as pz,
            tc.tile_pool(name="py", bufs=2, space="PSUM") as py,
        ):

            def load_ft(sg):
                """One dma_start for the whole sg slice (one SP issue slot);
                quarters are views into the one tile."""
                big = ftpool.tile([128, SGB], FP8, tag="ftbig", bufs=3)
                nc.sync.dma_start(big, feat[:, sg * SGB:(sg + 1) * SGB])
                return [big[:, c * QB:(c + 1) * QB] for c in range(NDR)]

            # ---- DMA priority order: everything the first block needs,
            # earliest-needed first (issue cost is ~0.6us per dma_start on
            # the SP sequencer, so keep the count low). ----
            w2 = cpool.tile([128, NT * NF], FP8)
            nc.sync.dma_start(w2[:, 0:NF], w2p[:, 0:NF])
            ft_bufs = {0: load_ft(0)}
            bia = cpool.tile([128, 2 * NT], F32)
            nc.sync.dma_start(bia, biases[:, :])
            # warm-up burst first on the GpSimd queue so the PE starts
            # immediately; it stays busy during the head DMA wait and the
            # HAM clock gate is at 8/8 when the first real matmuls issue.
            wt = cpool.tile([128, 128], BF16)
            nc.gpsimd.memset(wt, 0.0)
            ones = cpool.tile([128, 4], BF16)
            nc.gpsimd.memset(ones, 1.0)
            ones3 = ones.rearrange("p (u w) -> p u w", u=4)
            wp = pz.tile([128, SG], F32, tag="z")
            for _ in range(NWARM):
                nc.tensor.matmul(wp[:, 0:128], wt, wt, start=True, stop=True)

            # bulk loads ride the GpSimd SWDGE queue, HELD behind a dummy
            # read of the ft0 tile: the 16 DMA rings round-robin every
            # in-flight transfer, so without the hold the critical first
            # loads would finish no earlier than the whole input set.
            nc.gpsimd.dma_start(w2[:, NF:4 * NF], w2p[:, NF:4 * NF])
            ft0q = ft_bufs[0][0]
            big1 = ftpool.tile([128, SGB], FP8, tag="ftbig", bufs=3)
            pp = cpool.tile([128, NT * NCLS], BF16)
            nc.gpsimd.tensor_copy(big1[:, 0:4], ft0q[:, 0:4])
            nc.gpsimd.tensor_copy(w2[:, 4 * NF:4 * NF + 4], ft0q[:, 0:4])
            nc.gpsimd.tensor_copy(pp[:, 0:4], ft0q[:, 0:4])
            nc.gpsimd.dma_start(big1, feat[:, SGB:2 * SGB])
            ft_bufs[1] = [big1[:, c * QB:(c + 1) * QB] for c in range(NDR)]
            nc.gpsimd.dma_start(w2[:, 4 * NF:NT * NF], w2p[:, 4 * NF:NT * NF])
            nc.gpsimd.dma_start(pp, pip[:, :])

            def mm_group(t, ft):
                """One slot tile's 4-chunk DoubleRow accumulation -> zp."""
                zp = pz.tile([128, SG], F32, tag="z")
                for c in range(NDR):
                    wsl = w2[:, (t * KCH + 2 * c) * 128:
                             (t * KCH + 2 * c + 2) * 128]
                    nc.tensor.matmul(
                        zp,
                        wsl.rearrange("p (k s) -> p k s", k=2),
                        ft[c].rearrange("p (k b) -> p k b", k=2),
                        start=(c == 0), stop=(c == NDR - 1),
                        perf_mode=DR,
                    )
                return zp

            def t0_group(ft):
                """Tile-0 matmul + sigmoid -> d0 [slot, b]."""
                d0 = dpool.tile([128, SG], BF16, tag="d0")
                zp = mm_group(0, ft)
                nc.scalar.activation(
                    d0, zp, SIG, bias=bia[:, 0:1], scale=1.0 / WSCALE
                )
                return d0

            def xbar_transpose(tag, src):
                """[128, 512] -> chunkwise DMA-xbar transpose:
                dst[p, u*128+s] = src[s, u*128+p].  NOTE a DMA_TRANSPOSE
                drains every in-flight DMA, so SP-queue emission order
                decides what it ends up waiting for."""
                dst = tpool.tile([128, 512], BF16, tag=tag)
                nc.sync.dma_start_transpose(
                    dst.rearrange("p (u s) -> p u s", u=4), src
                )
                return dst

            def phase_a(t0T):
                """Tree levels 0-6 in [b, path] layout -> mu7 [b, 128]."""
                t03 = t0T.rearrange("p (u w) -> p u w", u=4)
                mu_prev = mupool.tile([128, 4 * 2], BF16, tag="muA1")
                mp3 = mu_prev.rearrange("p (u w) -> p u w", u=4)
                nc.vector.tensor_copy(mp3[:, :, 0:1], t03[:, :, 1:2])
                nc.vector.tensor_sub(mp3[:, :, 1:2], ones3, t03[:, :, 1:2])
                for l in range(1, 7):
                    w = 1 << l
                    mu_next = mupool.tile([128, 4 * 2 * w], BF16, tag=f"muA{l + 1}")
                    mn3 = mu_next.rearrange("p (u w) -> p u w", u=4)
                    nc.vector.tensor_mul(mn3[:, :, 0:w], mp3, t03[:, :, w:2 * w])
                    nc.vector.tensor_sub(mn3[:, :, w:2 * w], mp3, mn3[:, :, 0:w])
                    mu_prev, mp3 = mu_next, mn3
                return mu_prev

            # ---- prologue: tile0 chain for block 0 ----
            d0_cur = t0_group(ft_bufs[0])
            mu7_cur = phase_a(xbar_transpose("t0T", d0_cur))

            for sg in range(NSG):
                ft = ft_bufs.pop(sg)
                # SP order: m7T(sg) first (its inputs finished last block and
                # the older in-flight DMAs are done, so its drain is cheap),
                # THEN the next feature load
                m7T = xbar_transpose("m7T", mu7_cur)
                if sg + 2 < NSG:
                    ft_bufs[sg + 2] = load_ft(sg + 2)
                dsg = dpool.tile([128, 7 * SG], BF16, tag="d")

                def tile_mm(t):
                    zp = mm_group(t, ft)
                    nc.scalar.activation(
                        dsg[:, (t - 1) * SG:t * SG], zp, SIG,
                        bias=bia[:, t:t + 1], scale=1.0 / WSCALE,
                    )

                for t in range(1, 5):
                    tile_mm(t)
                # PE: tile0 group for the NEXT block (mid-block so its
                # sigmoid -> transpose -> phase A chain lands in time)
                d0_next = t0_group(ft_bufs[sg + 1]) if sg + 1 < NSG else None

                # DVE: tree levels 7-8 (m7T PSUM + d tiles 1-3)
                mu8 = mupool.tile([128, 2 * SG], BF16, tag="mu8")
                nc.vector.tensor_mul(mu8[:, 0:SG], m7T, dsg[:, 0:SG])
                nc.vector.tensor_sub(mu8[:, SG:2 * SG], m7T, mu8[:, 0:SG])
                mu9 = mupool.tile([128, 4 * SG], BF16, tag="mu9")
                for j1 in range(2):
                    nc.vector.tensor_mul(
                        mu9[:, j1 * SG:(j1 + 1) * SG],
                        mu8[:, j1 * SG:(j1 + 1) * SG],
                        dsg[:, (1 + j1) * SG:(2 + j1) * SG],
                    )
                    nc.vector.tensor_sub(
                        mu9[:, (2 + j1) * SG:(3 + j1) * SG],
                        mu8[:, j1 * SG:(j1 + 1) * SG],
                        mu9[:, j1 * SG:(j1 + 1) * SG],
                    )

                # SP: t0T(next) transpose (drain set: this block's own
                # feature load, already landed)
                t0T_next = (
                    xbar_transpose("t0T", d0_next) if d0_next is not None
                    else None
                )
                tile_mm(5)
                tile_mm(6)
                tile_mm(7)

                # DVE: tree level 9, ordered by d-tile availability
                mu10 = mupool.tile([128, 8 * SG], BF16, tag="mu10")
                for j2 in range(4):
                    nc.vector.tensor_mul(
                        mu10[:, j2 * SG:(j2 + 1) * SG],
                        mu9[:, j2 * SG:(j2 + 1) * SG],
                        dsg[:, (3 + j2) * SG:(4 + j2) * SG],
                    )
                    nc.vector.tensor_sub(
                        mu10[:, (4 + j2) * SG:(5 + j2) * SG],
                        mu9[:, j2 * SG:(j2 + 1) * SG],
                        mu10[:, j2 * SG:(j2 + 1) * SG],
                    )

                # PE: leaf matmuls, in mu10-readiness order
                yp = py.tile([NCLS, SG], F32, tag="y")
                leaf_order = [0, 4, 1, 5, 2, 6, 3, 7]
                for i, j3 in enumerate(leaf_order):
                    nc.tensor.matmul(
                        yp,
                        pp[:, j3 * NCLS:(j3 + 1) * NCLS],
                        mu10[:, j3 * SG:(j3 + 1) * SG],
                        start=(i == 0), stop=(i == 7),
                    )

                # DVE: phase A for block sg+1 (after mu10 so the FIFO never
                # stalls level-9 behind the t0T transpose)
                if t0T_next is not None:
                    mu7_cur = phase_a(t0T_next)

                # DVE copy + SP-queue store: the ACT queue carries ONLY
                # sigmoids, so a late leaf matmul can never delay the next
                # block's sigmoid chain through FIFO coupling
                ysb = opool.tile([NCLS, SG], F32, tag="ysb")
                nc.vector.tensor_copy(ysb, yp)
                nc.sync.dma_start(yT[:, sg * SG:(sg + 1) * SG], ysb)

    nc.finalize()
    return nc


_PROGRAM = None


def _get_program():
    global _PROGRAM
    if _PROGRAM is None:
        _PROGRAM = _build_program()
    return _PROGRAM


def kernel(features, mask, W, b, pi):
    global LAST_RESULT
    features = np.asarray(features, dtype=np.float32)
    mask = np.asarray(mask)
    W = np.asarray(W, dtype=np.float32)
    b = np.asarray(b, dtype=np.float32)
    pi = np.asarray(pi, dtype=np.float32)

    # one-hot selection -> host column gather; apply slot/leaf permutations
    idx = np.argmax(mask, axis=1)
    node = _node_of_slot()
    W2p = W[:, node] * WSCALE
    w2p_resh = np.ascontiguousarray(
        W2p.reshape(KCH, 128, NT, 128).transpose(1, 2, 0, 3).reshape(128, NT * NF)
    )
    w2p_fp8 = np.clip(w2p_resh, -240.0, 240.0).astype(ml_dtypes.float8_e4m3fn)
    b2 = b[node].astype(np.float32)
    bcols = b2.reshape(NT, 128).T                      # [128, NT]
    biases = np.ascontiguousarray(
        np.concatenate([bcols, -bcols], axis=1), dtype=np.float32
    )
    e = np.exp(pi.astype(np.float64) - pi.max(1, keepdims=True))
    probs = (e / e.sum(1, keepdims=True)).astype(np.float32)
    piP = probs[_leaf_of_row(), :]
    pip_resh = np.ascontiguousarray(
        piP.reshape(NT, 128, NCLS).transpose(1, 0, 2).reshape(128, NT * NCLS)
    ).astype(ml_dtypes.bfloat16)
    feat_fp8 = np.clip(features[:, idx], -240.0, 240.0).astype(
        ml_dtypes.float8_e4m3fn
    )

    nc = _get_program()
    in_maps = []
    for c in range(NCORES):
        xc = feat_fp8[c * BC:(c + 1) * BC]            # [BC, NF]
        # device layout [p, sg, k, b]: feat[p, ...] = x[sg*SG+b, 128k+p]
        fdev = np.ascontiguousarray(
            xc.reshape(NSG, SG, KCH, 128).transpose(3, 0, 2, 1).reshape(128, -1)
        )
        in_maps.append(
            {"feat": fdev, "w2p": w2p_fp8, "biases": biases, "pip": pip_resh}
        )
    res = run_bass_kernel_spmd(nc, in_maps, core_ids=list(range(NCORES)), **RUN_KWARGS)
    LAST_RESULT = res
    yT_full = np.concatenate([res.results[c]["yT"] for c in range(NCORES)], axis=1)
    return np.ascontiguousarray(yT_full.T)
